# revision 52
# baseline (speedup 1.0000x reference)
"""Trainium2 Bass kernel for nn_C4ByteTransformer (4-step carry-propagation
softmax table lookup).

Contract: kernel(**inputs) takes FULL inputs (a_emb[4,256], b_emb[4,256],
W1[514,131072], W2_sum[131072,256], W2_carry[131072,2]) and returns the
full [4,256] float32 output, running SPMD on 8 NeuronCores.

Math. The tables are the canonical carry-adder tables (validated exactly on
the host, with a numpy fallback otherwise): for entry k, with a=k//512,
b=(k//2)%256, c=k%2,
  scores[k] = a_i[a] + b_i[b] + carry[c]
so exp(10*(scores-2.5)) factorizes rank-1: w[a,b,c] ~ ea[a]*eb[b]*ec[c] with
ea=exp(10*a_i) etc.  Then with V0 = circular_conv(ea, eb)  (V0[m] =
sum_{a+b=m mod 256} ea*eb), r = ec1/ec0 = exp(10*(2*c1-1)):
  out_i      = (V0 + r*rot1(V0)) / (Sa*Sb*(1+r))
  c1_next    = (W0 + r*W1w)      / (Sa*Sb*(1+r))
  W0  = sum_{a+b>=256} ea*eb = (Sa-ea[0])*Sb - sum_t earev[t]*cum[t]   (t<255)
  W1w = sum_{a+b>=255} ea*eb = W0-analog via the shifted prefix sum
where cum is the prefix sum of eb and earev[t] = ea[255-t].  No W1/W2 bytes
ever touch the device: the kernel streams only ~540KB of host-expanded
Hankel layout of raw a/b embeddings, applies exp on-device (ScalarE), does
the convolution as 12 small float32r matmuls (N=256 full-rate), runs the
precision-critical W0/W1w/Z path in full fp32 on VectorE (prefix-scan +
fused multiply-reduce), the 4-step carry recurrence on 24 scalars, and one
DVE 32x32 block-transpose to diagonalize the per-step combine scalars.

All 8 cores run the identical program on identical inputs (no collectives);
core 0's output is returned.  Measured HW exec ~21.1us vs the 121.5us
streaming baseline (~5.8x); ~14us of that is fixed NEFF preamble/DMA-latency
/teardown overhead (an empty DMA-through kernel measures ~14.2us).
"""

import os

import numpy as np

N_CORES = 8
D = 256
NSTEP = 4
NE = 256 * 256 * 2
SCALE = 10.0
PAD = -300.0  # exp(10*PAD) == 0 in fp32 (and ACT clamps) -> exact zero lanes

_CACHE = {}

LAST_EXEC_TIME_NS = None


def _build_nc():
    import concourse.bacc as bacc
    import concourse.mybir as mybir
    import concourse.tile as tile

    f32 = mybir.dt.float32
    f32r = mybir.dt.float32r
    add = mybir.AluOpType.add
    subtract = mybir.AluOpType.subtract
    mult = mybir.AluOpType.mult
    bypass = mybir.AluOpType.bypass
    Exp = mybir.ActivationFunctionType.Exp

    nc = bacc.Bacc("TRN2", target_bir_lowering=False, debug=False,
                   num_devices=N_CORES)

    # small[i, 0:256] = a_emb[i][::-1]; small[i, 256:512] = b_emb[i];
    # small[i, 512:516] = I4[i] (one-hot selector for the flatten matmuls)
    small = nc.dram_tensor("small", [NSTEP, 2 * D + 4], f32,
                           kind="ExternalInput")
    # big[kp, 256*i + u] = b_emb[i][(1 + kp + u) % 256]        (cols 0:1024)
    # big[kp, 1024 + 4*(2i+kh) + c] = a_emb[i][255-(128*kh+kp)] if c==i
    #                                 else PAD                  (cols 1024:1056)
    big = nc.dram_tensor("big", [128, 1056], f32, kind="ExternalInput")
    out = nc.dram_tensor("out", [NSTEP, D], f32, kind="ExternalOutput")

    with tile.TileContext(nc) as tc:
        with (
            tc.tile_pool(name="sb", bufs=1) as sb,
            tc.tile_pool(name="ps", bufs=1, space="PSUM") as ps,
            tc.tile_pool(name="ps2", bufs=1, space="PSUM") as ps2,
            tc.tile_pool(name="ps3", bufs=1, space="PSUM") as ps3,
        ):
            # ---- resident tiles ----
            smallr = sb.tile([NSTEP, 2 * D + 4], f32)
            bigr = sb.tile([128, 1056], f32)
            aebe = sb.tile([NSTEP, 2 * D], f32)  # [., 0:256]=earev [., 256:]=eb
            accT = sb.tile([NSTEP, 1], f32)      # Sa per step (exp accum)
            bige = sb.tile([128, 1056], f32r)    # exp of big (f32r for the PE)
            cum = sb.tile([NSTEP, D], f32)       # prefix sum of eb
            junk = sb.tile([NSTEP, D], f32)      # fused multiply-reduce dump
            xs = sb.tile([NSTEP, 2], f32)        # X0, X1
            t2 = sb.tile([NSTEP, 1], f32)
            u2 = sb.tile([NSTEP, 1], f32)
            sc = sb.tile([NSTEP, 4], f32)        # (W1w, SaSb, W0, SaSb)
            mwarm = sb.tile([1, 1], f32)
            scp = sb.tile([1, 16], f32)          # sc flattened to partition 0
            bz128 = sb.tile([128, 1], f32)
            bz4 = sb.tile([NSTEP, 1], f32)
            bm10 = sb.tile([1, 1], f32)
            rs = sb.tile([1, 8], f32)            # r_0..r_4
            ndT = sb.tile([1, 2], f32)           # (num, den/20) per step
            fzT = sb.tile([32, 64], f32)         # row0: h at 0:4, g1 at 32:36
            fzD = sb.tile([32, 64], f32)         # block-transposed
            sbA = sb.tile([NSTEP, D], f32)       # final combine / output

            # ---- DMAs (all on Sync, small first: its 8KB must land before
            # the scan path can start; Scalar is busy with the ACT table
            # load anyway). big is split so each half's completion
            # semaphore fires earlier and the exp chunks pipeline. ----
            nc.sync.dma_start(smallr[:], small[:])
            nc.sync.dma_start(bigr[:, 512:1056], big[:, 512:1056])
            nc.sync.dma_start(bigr[:, 0:512], big[:, 0:512])

            nc.vector.memset(fzT[:], 0.0)
            nc.vector.memset(bz128[:], 0.0)
            nc.vector.memset(bz4[:], 0.0)
            nc.vector.memset(bm10[:], -10.0)
            nc.vector.memset(mwarm[:], -1.0)

            # Warmup ACT with no DMA dependency: walrus inserts the exp
            # table load right before the first ACTIVATE in the Scalar
            # stream, so this pulls the ~1.3us load off the DMA-gated
            # critical path. It also produces r_0 = exp(-10) (carry0=[1,0]).
            nc.scalar.activation(rs[0:1, 0:1], mwarm[:], Exp,
                                 bias=bm10[:], scale=0.0)

            # ---- exp of the small (precision-critical) operands; eb first
            # (it feeds the scan, the longest dependent chain), then ea with
            # the accumulator giving Sa for free ----
            earev = aebe[:, 0:D]
            ebf = aebe[:, D : 2 * D]
            nc.scalar.activation(ebf, smallr[:, D : 2 * D], Exp,
                                 bias=bz4[:], scale=SCALE)
            nc.scalar.activation(earev, smallr[:, 0:D], Exp,
                                 bias=bz4[:], scale=SCALE, accum_out=accT[:])

            # ---- exp of the Hankel block (feeds the PE), two chunks
            # pipelined behind the two big-DMA halves ----
            nc.scalar.activation(bige[:, 512:1056], bigr[:, 512:1056], Exp,
                                 bias=bz128[:], scale=SCALE)
            nc.scalar.activation(bige[:, 0:512], bigr[:, 0:512], Exp,
                                 bias=bz128[:], scale=SCALE)

            # ---- prefix sum of eb along the free dim ----
            nc.vector.tensor_tensor_scan(
                out=cum[:], data0=ebf, data1=ebf, initial=0.0,
                op0=add, op1=bypass,
            )

            # ---- X0 = sum_{t<255} earev[t]*cum[t];  X1 shifted.
            # (tensor_tensor_reduce wedges the device; scalar_tensor_tensor
            # with accum_out computes the same fused multiply-reduce.) ----
            nc.vector.scalar_tensor_tensor(
                out=junk[:, 0:255], in0=earev[:, 0:255], scalar=1.0,
                in1=cum[:, 0:255], op0=bypass, op1=mult,
                accum_out=xs[:, 0:1],
            )
            nc.vector.scalar_tensor_tensor(
                out=junk[:, 0:254], in0=earev[:, 1:255], scalar=1.0,
                in1=cum[:, 0:254], op0=bypass, op1=mult,
                accum_out=xs[:, 1:2],
            )

            # ---- assemble (W1w, SaSb, W0, SaSb) in fp32 ----
            Sb = cum[:, 255:256]
            ea0 = aebe[:, 255:256]      # = ea[0]
            eb255 = aebe[:, 511:512]
            nc.vector.scalar_tensor_tensor(
                out=t2[:], in0=accT[:], scalar=ea0, in1=Sb,
                op0=subtract, op1=mult)                           # (Sa-ea0)*Sb
            nc.vector.tensor_tensor(out=sc[:, 2:3], in0=t2[:], in1=xs[:, 0:1],
                                    op=subtract)                  # W0
            nc.vector.scalar_tensor_tensor(
                out=u2[:], in0=ea0, scalar=eb255, in1=xs[:, 1:2],
                op0=mult, op1=subtract)                           # ea0*eb255-X1
            nc.vector.tensor_tensor(out=sc[:, 0:1], in0=t2[:], in1=u2[:],
                                    op=add)                       # W1w
            # SaSb/20 into both slots (strided out AP): pre-scaling den by
            # 1/20 lets the chain ACT consume h = reciprocal(den/20) =
            # 20/den directly as its scale input (r' = exp(num*h - 10)),
            # dropping a DVE op per step.
            nc.vector.scalar_tensor_tensor(
                out=sc[:, 1:4:2], in0=accT[:].broadcast_to([NSTEP, 2]),
                scalar=0.05, in1=Sb.broadcast_to([NSTEP, 2]),
                op0=mult, op1=mult)

            # ---- flatten the per-step scalars onto partition 0: four
            # one-hot-column matmuls relocate row i of sc into cols 4i..4i+3
            # (partition-sum = relocation). Step 0 gets its own psum bank so
            # its copy-out (and thus the chain) can start while the other
            # three matmuls still run; the second copy hides under step 0's
            # ACT. ----
            scps0 = ps2.tile([1, 4], f32, tag="scps0")
            scpsR = ps3.tile([1, 12], f32, tag="scpsR")
            nc.tensor.matmul(
                scps0[:], lhsT=smallr[:, 2 * D : 2 * D + 1],
                rhs=sc[:], start=True, stop=True,
            )
            nc.vector.tensor_copy(out=scp[0:1, 0:4], in_=scps0[:])
            for i in range(1, NSTEP):
                nc.tensor.matmul(
                    scpsR[0:1, 4 * (i - 1) : 4 * (i - 1) + 4],
                    lhsT=smallr[:, 2 * D + i : 2 * D + i + 1],
                    rhs=sc[:], start=True, stop=True,
                )
            nc.vector.tensor_copy(out=scp[0:1, 4:16], in_=scpsR[:])

            # ---- 4-step carry recurrence on partition 0 ----
            # nd = (W1w, SaSb/20)*r + (W0, SaSb/20) = (num, den/20)
            # h = 20/den; r' = exp(num*h - 10) = exp(20*num/den - 10)
            for i in range(NSTEP):
                nc.vector.scalar_tensor_tensor(
                    out=ndT[:], in0=scp[0:1, 4 * i : 4 * i + 2],
                    scalar=rs[0:1, i : i + 1],
                    in1=scp[0:1, 4 * i + 2 : 4 * i + 4],
                    op0=mult, op1=add,
                )
                nc.vector.reciprocal(fzT[0:1, i : i + 1], ndT[0:1, 1:2])
                if i + 1 < NSTEP:
                    nc.scalar.activation(rs[0:1, i + 1 : i + 2],
                                         ndT[0:1, 0:1], Exp, bias=bm10[:],
                                         scale=fzT[0:1, i : i + 1])

            # g1_i = r_i * h_i / 20  (true F1_i/Z_i)
            nc.vector.scalar_tensor_tensor(
                out=fzT[0:1, 32:36], in0=rs[0:1, 0:4], scalar=0.05,
                in1=fzT[0:1, 0:4], op0=mult, op1=mult)

            # ---- diagonalize: block-transpose puts g0_i/g1_i on partition i
            nc.vector.transpose(fzD[:], fzT[:])

            # ---- main conv matmuls: psum[i, m] = V0_i[m] (float32r).
            # Steps 2,3 first: their Hankel chunk is exp'd first. ----
            pm = ps.tile([NSTEP, D], f32)
            for jj, j in enumerate([4, 5, 6, 7, 0, 1, 2, 3]):
                i, kh = divmod(j, 2)
                lhsT = bige[:, 1024 + 4 * j : 1024 + 4 * j + 4]
                if kh == 0:
                    nc.tensor.matmul(
                        pm[:], lhsT=lhsT,
                        rhs=bige[:, 256 * i : 256 * i + 256],
                        start=(jj == 0), stop=False,
                    )
                else:
                    nc.tensor.matmul(
                        pm[:, 0:128], lhsT=lhsT,
                        rhs=bige[:, 256 * i + 128 : 256 * i + 256],
                        start=False, stop=False,
                    )
                    nc.tensor.matmul(
                        pm[:, 128:256], lhsT=lhsT,
                        rhs=bige[:, 256 * i : 256 * i + 128],
                        start=False, stop=(jj == 7),
                    )

            # ---- combine: out_i = g0*V0 + g1*rot1(V0), g0 = h/20 ----
            nc.vector.tensor_scalar(
                out=sbA[:], in0=pm[:], scalar1=fzD[0:4, 0:1], scalar2=0.05,
                op0=mult, op1=mult,
            )
            nc.vector.scalar_tensor_tensor(
                out=sbA[:, 1:256], in0=pm[:, 0:255],
                scalar=fzD[0:4, 32:33], in1=sbA[:, 1:256],
                op0=mult, op1=add,
            )
            nc.vector.scalar_tensor_tensor(
                out=sbA[:, 0:1], in0=pm[:, 255:256],
                scalar=fzD[0:4, 32:33], in1=sbA[:, 0:1],
                op0=mult, op1=add,
            )
            nc.sync.dma_start(out[:], sbA[:])

    nc.compile()
    return nc


def _tables_ok(W1, W2_sum, W2_carry):
    """Exact equality against the canonical carry-adder tables."""
    k = np.arange(NE)
    a = k // 512
    b = (k // 2) % 256
    c = k % 2
    W1c = np.zeros((2 * D + 2, NE), dtype=np.float32)
    W1c[a, k] = 1.0
    W1c[D + b, k] = 1.0
    W1c[2 * D + c, k] = 1.0
    if W1.shape != W1c.shape or not np.array_equal(W1, W1c):
        return False
    total = a + b + c
    W2s = np.zeros((NE, D), dtype=np.float32)
    W2s[k, total & 255] = 1.0
    if W2_sum.shape != W2s.shape or not np.array_equal(W2_sum, W2s):
        return False
    W2c = np.zeros((NE, 2), dtype=np.float32)
    W2c[k, (total >= 256).astype(np.int64)] = 1.0
    return W2_carry.shape == W2c.shape and np.array_equal(W2_carry, W2c)


def _numpy_fallback(a_emb, b_emb, W1, W2_sum, W2_carry):
    carry = np.zeros(2, dtype=np.float64)
    carry[0] = 1.0
    outs = []
    W1 = W1.astype(np.float64)
    for i in range(NSTEP):
        x = np.concatenate([a_emb[i], b_emb[i], carry]).astype(np.float64)
        scores = x @ W1
        z = (scores - 2.5) * 10.0
        z -= z.max()
        w = np.exp(z)
        w /= w.sum()
        outs.append(w @ W2_sum.astype(np.float64))
        carry = w @ W2_carry.astype(np.float64)
    return np.stack(outs).astype(np.float32)


def _prep_inputs(a_emb, b_emb):
    small = np.empty((NSTEP, 2 * D + 4), dtype=np.float32)
    small[:, 0:D] = a_emb[:, ::-1]
    small[:, D : 2 * D] = b_emb
    small[:, 2 * D :] = np.eye(NSTEP, dtype=np.float32)

    big = np.full((128, 1056), PAD, dtype=np.float32)
    kp = np.arange(128)
    u = np.arange(D)
    gather = (1 + kp[:, None] + u[None, :]) % 256  # [128, 256]
    for i in range(NSTEP):
        big[:, 256 * i : 256 * i + 256] = b_emb[i][gather]
    for j in range(8):
        i, kh = divmod(j, 2)
        big[:, 1024 + 4 * j + i] = a_emb[i][255 - (128 * kh + kp)]

    in_map = {"small": np.ascontiguousarray(small),
              "big": np.ascontiguousarray(big)}
    return [in_map for _ in range(N_CORES)]


def kernel(a_emb, b_emb, W1, W2_sum, W2_carry):
    global LAST_EXEC_TIME_NS
    a_emb = np.asarray(a_emb, dtype=np.float32)
    b_emb = np.asarray(b_emb, dtype=np.float32)
    W1 = np.asarray(W1, dtype=np.float32)
    W2_sum = np.asarray(W2_sum, dtype=np.float32)
    W2_carry = np.asarray(W2_carry, dtype=np.float32)

    if not _tables_ok(W1, W2_sum, W2_carry):
        return _numpy_fallback(a_emb, b_emb, W1, W2_sum, W2_carry)

    from concourse.bass_utils import run_bass_kernel_spmd

    if "nc" not in _CACHE:
        _CACHE["nc"] = _build_nc()
    nc = _CACHE["nc"]

    in_maps = _prep_inputs(a_emb, b_emb)
    trace = os.environ.get("KERNEL_TRACE", "") == "1"
    res = run_bass_kernel_spmd(nc, in_maps, list(range(N_CORES)), trace=trace)
    LAST_EXEC_TIME_NS = res.exec_time_ns
    return np.asarray(res.results[0]["out"], dtype=np.float32)


# revision 54
# speedup vs baseline: 1.0754x; 1.0754x over previous
"""Trainium2 Bass kernel for nn_C4ByteTransformer (4-step carry-propagation
softmax table lookup).

Contract: kernel(**inputs) takes FULL inputs (a_emb[4,256], b_emb[4,256],
W1[514,131072], W2_sum[131072,256], W2_carry[131072,2]) and returns the
full [4,256] float32 output, running SPMD on 8 NeuronCores.

Math. The tables are the canonical carry-adder tables (validated exactly on
the host, with a numpy fallback otherwise): for entry k, with a=k//512,
b=(k//2)%256, c=k%2,
  scores[k] = a_i[a] + b_i[b] + carry[c]
so exp(10*(scores-2.5)) factorizes rank-1: w[a,b,c] ~ ea[a]*eb[b]*ec[c] with
ea=exp(10*a_i) etc.  Then with V0 = circular_conv(ea, eb)  (V0[m] =
sum_{a+b=m mod 256} ea*eb), r = ec1/ec0 = exp(10*(2*c1-1)):
  out_i      = (V0 + r*rot1(V0)) / (Sa*Sb*(1+r))
  c1_next    = (W0 + r*W1w)      / (Sa*Sb*(1+r))
  W0  = sum_{a+b>=256} ea*eb = (Sa-ea[0])*Sb - sum_t earev[t]*cum[t]   (t<255)
  W1w = sum_{a+b>=255} ea*eb = W0-analog via the shifted prefix sum
where cum is the prefix sum of eb and earev[t] = ea[255-t].  No W1/W2 bytes
ever touch the device: the kernel streams only ~540KB of host-expanded
Hankel layout of raw a/b embeddings, applies exp on-device (ScalarE), does
the convolution as 12 small float32r matmuls (N=256 full-rate), runs the
precision-critical W0/W1w/Z path in full fp32 on VectorE (prefix-scan +
fused multiply-reduce), the 4-step carry recurrence on 24 scalars, and one
DVE 32x32 block-transpose to diagonalize the per-step combine scalars.

All 8 cores run the identical program on identical inputs (no collectives);
core 0's output is returned.  Measured HW exec ~21.1us vs the 121.5us
streaming baseline (~5.8x); ~14us of that is fixed NEFF preamble/DMA-latency
/teardown overhead (an empty DMA-through kernel measures ~14.2us).
"""

import os

import numpy as np

N_CORES = 8
D = 256
NSTEP = 4
NE = 256 * 256 * 2
SCALE = 10.0
PAD = -300.0  # exp(10*PAD) == 0 in fp32 (and ACT clamps) -> exact zero lanes

_CACHE = {}

LAST_EXEC_TIME_NS = None


def _build_nc():
    import concourse.bacc as bacc
    import concourse.mybir as mybir
    import concourse.tile as tile

    f32 = mybir.dt.float32
    f32r = mybir.dt.float32r
    add = mybir.AluOpType.add
    subtract = mybir.AluOpType.subtract
    mult = mybir.AluOpType.mult
    bypass = mybir.AluOpType.bypass
    Exp = mybir.ActivationFunctionType.Exp

    nc = bacc.Bacc("TRN2", target_bir_lowering=False, debug=False,
                   num_devices=N_CORES)

    # small[i, 0:256] = a_emb[i][::-1]; small[i, 256:512] = b_emb[i];
    # small[i, 512:516] = I4[i] (one-hot selector for the flatten matmuls)
    small = nc.dram_tensor("small", [NSTEP, 2 * D + 4], f32,
                           kind="ExternalInput")
    # big[kp, 256*i + u] = b_emb[i][(1 + kp + u) % 256]        (cols 0:1024)
    # big[kp, 1024 + 4*(2i+kh) + c] = a_emb[i][255-(128*kh+kp)] if c==i
    #                                 else PAD                  (cols 1024:1056)
    big = nc.dram_tensor("big", [128, 1056], f32, kind="ExternalInput")
    out = nc.dram_tensor("out", [NSTEP, D], f32, kind="ExternalOutput")

    with tile.TileContext(nc) as tc:
        with (
            tc.tile_pool(name="sb", bufs=1) as sb,
            tc.tile_pool(name="ps", bufs=1, space="PSUM") as ps,
            tc.tile_pool(name="ps2", bufs=1, space="PSUM") as ps2,
        ):
            # ---- resident tiles ----
            smallr = sb.tile([NSTEP, 2 * D + 4], f32)
            bigr = sb.tile([128, 1056], f32)
            aebe = sb.tile([NSTEP, 2 * D], f32)  # [., 0:256]=earev [., 256:]=eb
            accT = sb.tile([NSTEP, 1], f32)      # Sa per step (exp accum)
            bige = sb.tile([128, 1056], f32r)    # exp of big (f32r for the PE)
            cum = sb.tile([NSTEP, D], f32)       # prefix sum of eb
            junk = sb.tile([NSTEP, D], f32)      # fused multiply-reduce dump
            xs = sb.tile([NSTEP, 2], f32)        # X0, X1
            t2 = sb.tile([NSTEP, 1], f32)
            u2 = sb.tile([NSTEP, 1], f32)
            sc = sb.tile([NSTEP, 4], f32)        # (W1w, SaSb, W0, SaSb)
            mwarm = sb.tile([1, 1], f32)
            scp = sb.tile([1, 16], f32)          # sc flattened to partition 0
            bz128 = sb.tile([128, 1], f32)
            bz4 = sb.tile([NSTEP, 1], f32)
            bm10 = sb.tile([1, 1], f32)
            rs = sb.tile([1, 8], f32)            # r_0..r_4
            ndT = sb.tile([1, 2], f32)           # (num, den/20) per step
            fzT = sb.tile([32, 64], f32)         # row0: h at 0:4, g1 at 32:36
            fzD = sb.tile([32, 64], f32)         # block-transposed
            sbA = sb.tile([NSTEP, D], f32)       # final combine / output

            # ---- DMAs (all on Sync, small first: its 8KB must land before
            # the scan path can start; Scalar is busy with the ACT table
            # load anyway). big is split so each half's completion
            # semaphore fires earlier and the exp chunks pipeline. ----
            nc.sync.dma_start(smallr[:], small[:])
            nc.sync.dma_start(bigr[:, 512:1056], big[:, 512:1056])
            nc.sync.dma_start(bigr[:, 0:512], big[:, 0:512])

            nc.vector.memset(fzT[:], 0.0)
            nc.vector.memset(bz128[:], 0.0)
            nc.vector.memset(bz4[:], 0.0)
            nc.vector.memset(bm10[:], -10.0)
            nc.vector.memset(mwarm[:], -1.0)

            # Warmup ACT with no DMA dependency: walrus inserts the exp
            # table load right before the first ACTIVATE in the Scalar
            # stream, so this pulls the ~1.3us load off the DMA-gated
            # critical path. It also produces r_0 = exp(-10) (carry0=[1,0]).
            nc.scalar.activation(rs[0:1, 0:1], mwarm[:], Exp,
                                 bias=bm10[:], scale=0.0)

            # ---- exp of the small (precision-critical) operands; eb first
            # (it feeds the scan, the longest dependent chain), then ea with
            # the accumulator giving Sa for free ----
            earev = aebe[:, 0:D]
            ebf = aebe[:, D : 2 * D]
            nc.scalar.activation(ebf, smallr[:, D : 2 * D], Exp,
                                 bias=bz4[:], scale=SCALE)
            nc.scalar.activation(earev, smallr[:, 0:D], Exp,
                                 bias=bz4[:], scale=SCALE, accum_out=accT[:])

            # ---- exp of the Hankel block (feeds the PE), two chunks
            # pipelined behind the two big-DMA halves ----
            nc.scalar.activation(bige[:, 512:1056], bigr[:, 512:1056], Exp,
                                 bias=bz128[:], scale=SCALE)
            nc.scalar.activation(bige[:, 0:512], bigr[:, 0:512], Exp,
                                 bias=bz128[:], scale=SCALE)

            # ---- prefix sum of eb along the free dim ----
            nc.vector.tensor_tensor_scan(
                out=cum[:], data0=ebf, data1=ebf, initial=0.0,
                op0=add, op1=bypass,
            )

            # ---- X0 = sum_{t<255} earev[t]*cum[t];  X1 shifted.
            # (tensor_tensor_reduce wedges the device; scalar_tensor_tensor
            # with accum_out computes the same fused multiply-reduce.) ----
            nc.vector.scalar_tensor_tensor(
                out=junk[:, 0:255], in0=earev[:, 0:255], scalar=1.0,
                in1=cum[:, 0:255], op0=bypass, op1=mult,
                accum_out=xs[:, 0:1],
            )
            nc.vector.scalar_tensor_tensor(
                out=junk[:, 0:254], in0=earev[:, 1:255], scalar=1.0,
                in1=cum[:, 0:254], op0=bypass, op1=mult,
                accum_out=xs[:, 1:2],
            )

            # ---- assemble (W1w, SaSb, W0, SaSb) in fp32 ----
            Sb = cum[:, 255:256]
            ea0 = aebe[:, 255:256]      # = ea[0]
            eb255 = aebe[:, 511:512]
            nc.vector.scalar_tensor_tensor(
                out=t2[:], in0=accT[:], scalar=ea0, in1=Sb,
                op0=subtract, op1=mult)                           # (Sa-ea0)*Sb
            nc.vector.tensor_tensor(out=sc[:, 2:3], in0=t2[:], in1=xs[:, 0:1],
                                    op=subtract)                  # W0
            nc.vector.scalar_tensor_tensor(
                out=u2[:], in0=ea0, scalar=eb255, in1=xs[:, 1:2],
                op0=mult, op1=subtract)                           # ea0*eb255-X1
            nc.vector.tensor_tensor(out=sc[:, 0:1], in0=t2[:], in1=u2[:],
                                    op=add)                       # W1w
            # SaSb/20 into both slots (strided out AP): pre-scaling den by
            # 1/20 lets the chain ACT consume h = reciprocal(den/20) =
            # 20/den directly as its scale input (r' = exp(num*h - 10)),
            # dropping a DVE op per step.
            nc.vector.scalar_tensor_tensor(
                out=sc[:, 1:4:2], in0=accT[:].broadcast_to([NSTEP, 2]),
                scalar=0.05, in1=Sb.broadcast_to([NSTEP, 2]),
                op0=mult, op1=mult)

            # ---- flatten the per-step scalars onto partition 0: four
            # one-hot-column matmuls relocate row i of sc into cols 4i..4i+3
            # of a single psum bank (partition-sum = relocation). ----
            scps = ps2.tile([1, 16], f32, tag="scps")
            for i in range(NSTEP):
                nc.tensor.matmul(
                    scps[0:1, 4 * i : 4 * i + 4],
                    lhsT=smallr[:, 2 * D + i : 2 * D + i + 1],
                    rhs=sc[:], start=True, stop=True,
                )
            nc.vector.tensor_copy(out=scp[:], in_=scps[:])

            # ---- 4-step carry recurrence on partition 0 ----
            # nd = (W1w, SaSb/20)*r + (W0, SaSb/20) = (num, den/20)
            # h = 20/den; r' = exp(num*h - 10) = exp(20*num/den - 10)
            for i in range(NSTEP):
                nc.vector.scalar_tensor_tensor(
                    out=ndT[:], in0=scp[0:1, 4 * i : 4 * i + 2],
                    scalar=rs[0:1, i : i + 1],
                    in1=scp[0:1, 4 * i + 2 : 4 * i + 4],
                    op0=mult, op1=add,
                )
                nc.vector.reciprocal(fzT[0:1, i : i + 1], ndT[0:1, 1:2])
                if i + 1 < NSTEP:
                    nc.scalar.activation(rs[0:1, i + 1 : i + 2],
                                         ndT[0:1, 0:1], Exp, bias=bm10[:],
                                         scale=fzT[0:1, i : i + 1])

            # g1_i = r_i * h_i / 20  (true F1_i/Z_i)
            nc.vector.scalar_tensor_tensor(
                out=fzT[0:1, 32:36], in0=rs[0:1, 0:4], scalar=0.05,
                in1=fzT[0:1, 0:4], op0=mult, op1=mult)

            # ---- diagonalize: block-transpose puts g0_i/g1_i on partition i
            nc.vector.transpose(fzD[:], fzT[:])

            # ---- main conv matmuls: psum[i, m] = V0_i[m] (float32r).
            # Steps 2,3 first: their Hankel chunk is exp'd first. ----
            pm = ps.tile([NSTEP, D], f32)
            for jj, j in enumerate([4, 5, 6, 7, 0, 1, 2, 3]):
                i, kh = divmod(j, 2)
                lhsT = bige[:, 1024 + 4 * j : 1024 + 4 * j + 4]
                if kh == 0:
                    nc.tensor.matmul(
                        pm[:], lhsT=lhsT,
                        rhs=bige[:, 256 * i : 256 * i + 256],
                        start=(jj == 0), stop=False,
                    )
                else:
                    nc.tensor.matmul(
                        pm[:, 0:128], lhsT=lhsT,
                        rhs=bige[:, 256 * i + 128 : 256 * i + 256],
                        start=False, stop=False,
                    )
                    nc.tensor.matmul(
                        pm[:, 128:256], lhsT=lhsT,
                        rhs=bige[:, 256 * i : 256 * i + 128],
                        start=False, stop=(jj == 7),
                    )

            # ---- combine: out_i = g0*V0 + g1*rot1(V0), g0 = h/20 ----
            nc.vector.tensor_scalar(
                out=sbA[:], in0=pm[:], scalar1=fzD[0:4, 0:1], scalar2=0.05,
                op0=mult, op1=mult,
            )
            nc.vector.scalar_tensor_tensor(
                out=sbA[:, 1:256], in0=pm[:, 0:255],
                scalar=fzD[0:4, 32:33], in1=sbA[:, 1:256],
                op0=mult, op1=add,
            )
            nc.vector.scalar_tensor_tensor(
                out=sbA[:, 0:1], in0=pm[:, 255:256],
                scalar=fzD[0:4, 32:33], in1=sbA[:, 0:1],
                op0=mult, op1=add,
            )
            nc.sync.dma_start(out[:], sbA[:])

    nc.compile()
    return nc


def _tables_ok(W1, W2_sum, W2_carry):
    """Exact equality against the canonical carry-adder tables."""
    k = np.arange(NE)
    a = k // 512
    b = (k // 2) % 256
    c = k % 2
    W1c = np.zeros((2 * D + 2, NE), dtype=np.float32)
    W1c[a, k] = 1.0
    W1c[D + b, k] = 1.0
    W1c[2 * D + c, k] = 1.0
    if W1.shape != W1c.shape or not np.array_equal(W1, W1c):
        return False
    total = a + b + c
    W2s = np.zeros((NE, D), dtype=np.float32)
    W2s[k, total & 255] = 1.0
    if W2_sum.shape != W2s.shape or not np.array_equal(W2_sum, W2s):
        return False
    W2c = np.zeros((NE, 2), dtype=np.float32)
    W2c[k, (total >= 256).astype(np.int64)] = 1.0
    return W2_carry.shape == W2c.shape and np.array_equal(W2_carry, W2c)


def _numpy_fallback(a_emb, b_emb, W1, W2_sum, W2_carry):
    carry = np.zeros(2, dtype=np.float64)
    carry[0] = 1.0
    outs = []
    W1 = W1.astype(np.float64)
    for i in range(NSTEP):
        x = np.concatenate([a_emb[i], b_emb[i], carry]).astype(np.float64)
        scores = x @ W1
        z = (scores - 2.5) * 10.0
        z -= z.max()
        w = np.exp(z)
        w /= w.sum()
        outs.append(w @ W2_sum.astype(np.float64))
        carry = w @ W2_carry.astype(np.float64)
    return np.stack(outs).astype(np.float32)


def _prep_inputs(a_emb, b_emb):
    small = np.empty((NSTEP, 2 * D + 4), dtype=np.float32)
    small[:, 0:D] = a_emb[:, ::-1]
    small[:, D : 2 * D] = b_emb
    small[:, 2 * D :] = np.eye(NSTEP, dtype=np.float32)

    big = np.full((128, 1056), PAD, dtype=np.float32)
    kp = np.arange(128)
    u = np.arange(D)
    gather = (1 + kp[:, None] + u[None, :]) % 256  # [128, 256]
    for i in range(NSTEP):
        big[:, 256 * i : 256 * i + 256] = b_emb[i][gather]
    for j in range(8):
        i, kh = divmod(j, 2)
        big[:, 1024 + 4 * j + i] = a_emb[i][255 - (128 * kh + kp)]

    in_map = {"small": np.ascontiguousarray(small),
              "big": np.ascontiguousarray(big)}
    return [in_map for _ in range(N_CORES)]


def kernel(a_emb, b_emb, W1, W2_sum, W2_carry):
    global LAST_EXEC_TIME_NS
    a_emb = np.asarray(a_emb, dtype=np.float32)
    b_emb = np.asarray(b_emb, dtype=np.float32)
    W1 = np.asarray(W1, dtype=np.float32)
    W2_sum = np.asarray(W2_sum, dtype=np.float32)
    W2_carry = np.asarray(W2_carry, dtype=np.float32)

    if not _tables_ok(W1, W2_sum, W2_carry):
        return _numpy_fallback(a_emb, b_emb, W1, W2_sum, W2_carry)

    from concourse.bass_utils import run_bass_kernel_spmd

    if "nc" not in _CACHE:
        _CACHE["nc"] = _build_nc()
    nc = _CACHE["nc"]

    in_maps = _prep_inputs(a_emb, b_emb)
    trace = os.environ.get("KERNEL_TRACE", "") == "1"
    res = run_bass_kernel_spmd(nc, in_maps, list(range(N_CORES)), trace=trace)
    LAST_EXEC_TIME_NS = res.exec_time_ns
    return np.asarray(res.results[0]["out"], dtype=np.float32)
